# revision 6
# baseline (speedup 1.0000x reference)
"""Trainium2 Bass kernel for the gated-attention module (8 NeuronCores, SPMD).

Module math (per reference):
    qsig = sigmoid(qs); ksig = sigmoid(ks_p)
    vsig = sigmoid(f)*tanh(c),  (c,f) = split(sigmoid(vs) @ vq_w.T + vq_b)
    q = qsig * LN(query @ ql_w.T + ql_b)        [S,B,H]
    k = ksig * key ; v = vsig * value
    out[q,b,:] = softmax(q_h . k_h / sqrt(H)) @ v_h   (per head h)

Kernel strategy:
  - Shard (batch, query-block): core = b*4 + qc handles query rows
    [qc*512:(qc+1)*512] of batch b, with full K/V for that batch.
  - Host-side constant folding of the tiny gate vectors (pure functions of
    the module *parameters*, no data dependence):
        G  = qsig*ksig*ln_g/sqrt(H); Bv = qsig*ksig*ln_b/sqrt(H); vsig
    so on-device  q_eff = norm(y)*G + Bv,  scores = q_eff . key  (no key
    gating needed),  out = vsig * (P @ value).
  - bf16 on-device matmul operands (inputs pre-cast on host); fp32 psum
    accumulation; fp32 LN statistics and output.
  - Scores are computed transposed (k on partitions) so softmax's P feeds
    the PV matmul directly with no P transpose; the softmax denominator
    comes for free from a ones-column appended to V. exp() is safe without
    max-subtraction here because |scores| <~ 0.4 (LN + sigmoid-gate/32
    scaling keeps q tiny; exp overflow would need |s|>88).
"""

import sys

sys.path.insert(0, "/opt/trn_rl_repo")

import numpy as np
import ml_dtypes

S = 2048
B = 2
H = 1024
H2 = 2 * H
NH = 16
HD = 64
TQ = S // 4  # 512 query rows per core
NKC = S // 128  # 16 k-chunks
SCALE = float(np.sqrt(H))
EPS = 1e-12

_CACHE = {}


def _build_bass():
    import concourse.bacc as bacc
    import concourse.bass as bass
    import concourse.tile as tile
    from concourse import mybir
    from concourse.masks import make_identity

    f32 = mybir.dt.float32
    bf16 = mybir.dt.bfloat16
    AF = mybir.ActivationFunctionType
    ALU = mybir.AluOpType

    nc = bacc.Bacc(None, target_bir_lowering=False)

    q_d = nc.dram_tensor("q", [TQ, H2], bf16, kind="ExternalInput")
    k_d = nc.dram_tensor("k", [S, H], bf16, kind="ExternalInput")
    w_d = nc.dram_tensor("w", [H, H2], bf16, kind="ExternalInput")
    v_d = nc.dram_tensor("vaug", [NKC, 128, NH, HD + 1], bf16, kind="ExternalInput")
    qlb_d = nc.dram_tensor("qlb", [H], f32, kind="ExternalInput")
    g_d = nc.dram_tensor("gvec", [H], f32, kind="ExternalInput")
    bv_d = nc.dram_tensor("bvec", [H], f32, kind="ExternalInput")
    vs_d = nc.dram_tensor("vsig", [H], f32, kind="ExternalInput")
    out_d = nc.dram_tensor("out", [TQ, H], f32, kind="ExternalOutput")

    def bcast(dram_handle):
        # replicate a [H] dram vector across all 128 partitions
        ap = dram_handle[:]
        return bass.AP(tensor=ap.tensor, offset=ap.offset, ap=[[0, 128], [1, H]])

    with tile.TileContext(nc) as tc:
        with tc.tile_pool(name="persist", bufs=1) as persist:
            id_bf = persist.tile([128, 128], bf16)
            make_identity(nc, id_bf)
            id_f32 = persist.tile([128, 128], f32)
            make_identity(nc, id_f32)
            eps_t = persist.tile([128, 1], f32)
            nc.vector.memset(eps_t[:], EPS)

            qlb_r = persist.tile([128, H], f32)
            g_r = persist.tile([128, H], f32)
            bv_r = persist.tile([128, H], f32)
            vs_r = persist.tile([128, H], f32)
            nc.gpsimd.dma_start(out=qlb_r[:], in_=bcast(qlb_d))
            nc.gpsimd.dma_start(out=g_r[:], in_=bcast(g_d))
            nc.gpsimd.dma_start(out=bv_r[:], in_=bcast(bv_d))
            nc.gpsimd.dma_start(out=vs_r[:], in_=bcast(vs_d))

            # K^T: kt[:, dc, :] = k[:, dc*128:(dc+1)*128].T   (xbar transpose)
            kt = persist.tile([128, 8, S], bf16)
            for dc in range(8):
                nc.sync.dma_start(
                    out=kt[:, dc, :],
                    in_=k_d[:, dc * 128 : (dc + 1) * 128],
                    transpose=True,
                )

            # V (+ ones column): vsb[p, kc, h, m] = vaug[kc, p, h, m]
            vsb = persist.tile([128, NKC, NH, HD + 1], bf16)
            nc.gpsimd.dma_start(
                out=vsb[:], in_=v_d[:].rearrange("c p h m -> p c h m")
            )

            # q_eff^T accumulates here: qeT[:, oc, tc*128:...] (bf16)
            qeT = persist.tile([128, 8, TQ], bf16)
            # final output staging, one tile per 128-row query block
            outsb = [
                persist.tile([128, H], f32, name=f"outsb{i}", tag=f"outsb{i}")
                for i in range(4)
            ]

            # ---------------- phase 1+2: q_linear + LayerNorm ----------------
            with tc.tile_pool(name="ph2in", bufs=1) as ph2in:
                qt = ph2in.tile([128, 16, TQ], bf16)
                for ic in range(16):
                    nc.sync.dma_start(
                        out=qt[:, ic, :],
                        in_=q_d[:, ic * 128 : (ic + 1) * 128],
                        transpose=True,
                    )
                wt = ph2in.tile([128, 16, H], bf16)
                for ic in range(16):
                    nc.sync.dma_start(
                        out=wt[:, ic, :],
                        in_=w_d[:, ic * 128 : (ic + 1) * 128],
                        transpose=True,
                    )

                with (
                    tc.tile_pool(name="ylin", bufs=2, space="PSUM") as ylin,
                    tc.tile_pool(name="tpq", bufs=2, space="PSUM") as tpq,
                    tc.tile_pool(name="ysb", bufs=2) as ysb_pool,
                    tc.tile_pool(name="stats", bufs=4) as stats_pool,
                ):
                    for tc4 in range(4):
                        y_ps = ylin.tile([128, 2, 512], f32)
                        for ic in range(16):
                            lhsT = qt[:, ic, tc4 * 128 : (tc4 + 1) * 128]
                            for oc in range(2):
                                nc.tensor.matmul(
                                    y_ps[:, oc, :],
                                    lhsT=lhsT,
                                    rhs=wt[:, ic, oc * 512 : (oc + 1) * 512],
                                    start=(ic == 0),
                                    stop=(ic == 15),
                                )
                        y_sb = ysb_pool.tile([128, H], f32)
                        nc.vector.tensor_add(
                            y_sb[:], y_ps[:].rearrange("p a b -> p (a b)"), qlb_r[:]
                        )
                        st = stats_pool.tile([128, 2, 6], f32)
                        nc.vector.bn_stats(st[:, 0, :], y_sb[:, 0:512])
                        nc.vector.bn_stats(st[:, 1, :], y_sb[:, 512:1024])
                        mv = stats_pool.tile([128, 2], f32)
                        nc.vector.bn_aggr(mv[:], st[:])
                        lv = stats_pool.tile([128, 1], f32)
                        nc.scalar.activation(lv[:], mv[:, 1:2], AF.Ln, bias=eps_t[:])
                        rst = stats_pool.tile([128, 1], f32)
                        nc.scalar.activation(rst[:], lv[:], AF.Exp, scale=-0.5)
                        # normalize in place, then *G, then +Bv (cast to bf16)
                        nc.vector.tensor_scalar(
                            out=y_sb[:],
                            in0=y_sb[:],
                            scalar1=mv[:, 0:1],
                            scalar2=rst[:],
                            op0=ALU.subtract,
                            op1=ALU.mult,
                        )
                        nc.vector.tensor_mul(y_sb[:], y_sb[:], g_r[:])
                        qe = ysb_pool.tile([128, H], bf16, tag="qe")
                        nc.vector.tensor_add(qe[:], y_sb[:], bv_r[:])
                        # transpose q_eff into qeT
                        for oc8 in range(8):
                            tp = tpq.tile([128, 128], bf16)
                            nc.tensor.transpose(
                                tp[:],
                                qe[:, oc8 * 128 : (oc8 + 1) * 128],
                                id_bf[:],
                            )
                            nc.vector.tensor_copy(
                                qeT[:, oc8, tc4 * 128 : (tc4 + 1) * 128], tp[:]
                            )

            # ---------------- phase 3: attention per head ----------------
            with (
                tc.tile_pool(name="sc", bufs=2, space="PSUM") as sc_pool,
                tc.tile_pool(name="pv", bufs=2, space="PSUM") as pv_pool,
                tc.tile_pool(name="tp2", bufs=2, space="PSUM") as tp2_pool,
                tc.tile_pool(name="pt", bufs=3) as pt_pool,
                tc.tile_pool(name="pvsb", bufs=2) as pvsb_pool,
                tc.tile_pool(name="rec", bufs=4) as rec_pool,
            ):
                for h in range(NH):
                    oc = h // 2
                    poff = 64 * (h % 2)
                    pv_ps = pv_pool.tile([65, 512], f32)
                    for g in range(8):
                        sc = sc_pool.tile([128, 2, 512], f32)
                        for j in range(2):
                            kc = 2 * g + j
                            nc.tensor.matmul(
                                sc[:, j, :],
                                lhsT=kt[poff : poff + 64, oc, kc * 128 : (kc + 1) * 128],
                                rhs=qeT[poff : poff + 64, oc, :],
                                start=True,
                                stop=True,
                            )
                        pt = pt_pool.tile([128, 2, 512], bf16)
                        nc.scalar.activation(
                            pt[:].rearrange("p a b -> p (a b)"),
                            sc[:].rearrange("p a b -> p (a b)"),
                            AF.Exp,
                        )
                        for j in range(2):
                            kc = 2 * g + j
                            nc.tensor.matmul(
                                pv_ps[:],
                                lhsT=vsb[:, kc, h, :],
                                rhs=pt[:, j, :],
                                start=(g == 0 and j == 0),
                                stop=(g == 7 and j == 1),
                            )
                    pv_sb = pvsb_pool.tile([65, 512], f32)
                    nc.vector.tensor_copy(pv_sb[:], pv_ps[:])
                    for qc in range(4):
                        tp2 = tp2_pool.tile([128, 65], f32)
                        nc.tensor.transpose(
                            tp2[:],
                            pv_sb[:, qc * 128 : (qc + 1) * 128],
                            id_f32[0:65, 0:65],
                        )
                        rec = rec_pool.tile([128, 1], f32)
                        nc.vector.reciprocal(rec[:], tp2[:, 64:65])
                        nc.vector.tensor_scalar_mul(
                            outsb[qc][:, h * HD : (h + 1) * HD],
                            in0=tp2[:, 0:64],
                            scalar1=rec[:],
                        )
                for qc in range(4):
                    nc.vector.tensor_mul(outsb[qc][:], outsb[qc][:], vs_r[:])
                    nc.sync.dma_start(
                        out=out_d[qc * 128 : (qc + 1) * 128, :], in_=outsb[qc][:]
                    )

    nc.compile()
    return nc


def _host_prep(query, key, value, qs, ks_p, vs, vq_w, vq_b, ql_w, ql_b, ln_g, ln_b):
    """Fold the gate-parameter math on host; build per-core device inputs."""
    bf16 = ml_dtypes.bfloat16

    def sig(x):
        return 1.0 / (1.0 + np.exp(-x.astype(np.float64)))

    qsig = sig(qs).reshape(H)
    ksig = sig(ks_p).reshape(H)
    hg = sig(vs).reshape(H) @ vq_w.astype(np.float64).T + vq_b.astype(np.float64)
    c, f = hg[:H], hg[H:]
    vsig = (1.0 / (1.0 + np.exp(-f))) * np.tanh(c)
    gg = qsig * ksig / SCALE
    G = (gg * ln_g.astype(np.float64)).astype(np.float32)
    Bv = (gg * ln_b.astype(np.float64)).astype(np.float32)
    vsig = vsig.astype(np.float32)
    qlb = ql_b.astype(np.float32)

    w_bf = np.ascontiguousarray(ql_w.astype(bf16))

    in_maps = []
    for core in range(8):
        b, qc = core // 4, core % 4
        q_sh = np.ascontiguousarray(
            query[qc * TQ : (qc + 1) * TQ, b, :].astype(bf16)
        )
        k_sh = np.ascontiguousarray(key[:, b, :].astype(bf16))
        v_b = value[:, b, :].reshape(NKC, 128, NH, HD)
        vaug = np.concatenate(
            [v_b, np.ones((NKC, 128, NH, 1), np.float32)], axis=-1
        ).astype(bf16)
        in_maps.append(
            {
                "q": q_sh,
                "k": k_sh,
                "w": w_bf,
                "vaug": np.ascontiguousarray(vaug),
                "qlb": qlb,
                "gvec": G,
                "bvec": Bv,
                "vsig": vsig,
            }
        )
    return in_maps


def kernel(**inputs):
    from concourse.bass_utils import run_bass_kernel_spmd

    if "nc" not in _CACHE:
        _CACHE["nc"] = _build_bass()
    nc = _CACHE["nc"]

    in_maps = _host_prep(**inputs)
    res = run_bass_kernel_spmd(nc, in_maps, core_ids=list(range(8)))

    out = np.empty((S, B, H), np.float32)
    for core in range(8):
        b, qc = core // 4, core % 4
        out[qc * TQ : (qc + 1) * TQ, b, :] = res.results[core]["out"]
    return out
